# revision 14
# baseline (speedup 1.0000x reference)
"""Trainium2 Bass kernel for nn_Attention_54580444397738 (gnn_message_passing).

Math per batch b (B=8, N=128, H=256, C=16):
  proj         = local @ W_apair                                     [N, H]
  pre[i,j,:]   = proj[i,:] + proj[j,:] + binary[i,j,:] @ W_binary
                 + b_apair + b_binary                                [N, N, H]
  score[i,j]   = sigmoid(relu(pre[i,j,:]) . W_att + b_att)           [N, N]
  glob         = score @ local                                       [N, H]
  local_pair [i,j,:] = local[i,:] + local[j,:]                       (output 1)
  global_pair[i,j,:] = glob[i,:]  + glob[j,:]                        (output 2)

Sharding: data-parallel over batch B across the 8 cores (1 batch per core).
The outputs (2 x 16 MB fp32 per core) dominate -> memory-bound.

v2 design (i-partition output layout):
  - Output tiles staged as [i(part)=128, JB*H] and stored with
    out[:, j0:j0+JB, :] DMAs: each partition writes one contiguous
    JB-KB chunk (measured 335 GB/s vs 272 GB/s for the j-partition
    1KB-descriptor pattern of v1).
  - tile[i, (j,h)] = x[i,h] (DVE free-dim broadcast of the SBUF-resident
    x) + x[j,h] (single f32r ones-row matmul broadcast into PSUM, rows
    fetched flat from DRAM at partitions {0,32,64,96}).  f32r rounding
    ~1.2e-4 relative, far inside the 2e-2 gate; no compensated pairs.
  - Attention runs i-oriented: binary loads contiguously as
    [i, (j, c16)] (8KB descriptors), GPSIMD pads c16->c32 into
    binp[i, (j, c32)] with a ones lane at c=16, PE transposes 128-col
    blocks to binT2[(j4, c32), i] (32-aligned j-slices), and one matmul
    per j both contracts W_binary and adds (proj[j] + biases) via the
    ones lane against wxAll (W_binary replicated at partitions
    {0,32,64,96}+c, row 32l+16 = proj[4k+l]+b).  identR@projW adds the
    proj[i] term; ACT applies relu; DVE affine_mul_reduce produces
    logits[i, j]; sigmoid(+b_att) -> score; PE transpose -> scoreT;
    fp32 matmul -> glob[i, :] in PSUM.
  - Known HW quirks honored: f32r producers write f32r dtype; matmul
    operands need 32-aligned base partitions; matmul order within a
    PSUM accumulation group is kept fixed.
"""

import numpy as np

B, N, H, BIN = 8, 128, 256, 16
NCORES = 8
CPAD = 32        # c dim padded 16 -> 32 so transposed blocks land 32-aligned
JB = 8           # j's per staged output flush (1 MB DMA, 8KB descriptors)

SKIP_ATTN = False  # probe knob: drop attention/score work (wrong gp values)
ATTN_LEVEL = 3     # probe knob: 1=transposes only, 2=+pre matmuls/relu/affine, 3=full

# Output-tile variants per pair-slot, chosen to balance engines:
#   D: PE row-broadcast into PSUM + DVE add (xSb free-dim broadcast)
#   E: PE identR-broadcast + row-broadcast in PSUM + ACT copy
#   G: GPSIMD partition_broadcast + GPSIMD add (SBUF only, no PSUM)
# Phase 1 runs attention (PE/DVE/ACT-heavy), so its slots lean on E/G while
# attention is live, then D once it drains.  ATTN_SLOTS = slot index where
# attention work is done (density ~64/ATTN_SLOTS pairs per slot).
ATTN_SLOTS = 56
P1_LIVE = "EG"    # slot p < ATTN_SLOTS: P1_LIVE[p % len]
P1_TAIL = "DDED"  # slot p >= ATTN_SLOTS
P2_PAT = "DDED"   # phase 2 (no G: gRows are staged for phase-1 odd slots only)

_cache = {}


def _body(tc, io, reps=1):
    import concourse.bass as bass
    import concourse.mybir as mybir
    from concourse.masks import make_identity
    from contextlib import ExitStack, nullcontext

    nc = tc.nc
    ts = bass.ts
    f32 = mybir.dt.float32
    f32r = mybir.dt.float32r
    Relu = mybir.ActivationFunctionType.Relu
    Sigmoid = mybir.ActivationFunctionType.Sigmoid

    local_d, binary_d, wap_d, bap_d, wbin_d, bbin_d, watt_d, batt_d, lp_d, gp_d = io

    ctx = ExitStack()
    with ctx:
        persist = ctx.enter_context(tc.tile_pool(name="persist", bufs=1))
        att2p = ctx.enter_context(tc.tile_pool(name="att2p", bufs=3))
        stagep = ctx.enter_context(tc.tile_pool(name="stagep", bufs=2))
        prep = ctx.enter_context(tc.tile_pool(name="prep", bufs=3, space="PSUM"))
        bcp = ctx.enter_context(tc.tile_pool(name="bcp", bufs=3, space="PSUM"))
        outpp = ctx.enter_context(tc.tile_pool(name="outpp", bufs=2, space="PSUM"))
        dramp = ctx.enter_context(tc.tile_pool(name="dramp", bufs=1, space="DRAM"))

        # timing builds wrap the whole body in a device-side loop
        loop = tc.For_i(0, reps, 1) if reps > 1 else nullcontext()
        ctx.enter_context(loop)

        # ---------------- persistent setup ----------------
        identity = persist.tile([128, 128], f32, tag="identity")
        make_identity(nc, identity)
        identR = persist.tile([128, 128], f32r, tag="identR")
        nc.vector.tensor_copy(out=identR, in_=identity)
        onesF = persist.tile([128, 128], f32, tag="onesF")
        nc.gpsimd.memset(onesF, 1.0)
        onesT = persist.tile([128, 128], f32r, tag="onesT")
        nc.vector.tensor_copy(out=onesT, in_=onesF)

        localSb = persist.tile([N, H], f32, tag="localSb")
        nc.sync.dma_start(out=localSb, in_=local_d)

        # flat local rows for the phase-1 broadcasts: row q holds
        # local[32q:32q+32, :] flattened, parked at partition 32q.
        flatL = persist.tile([97, 32 * H], f32r, tag="flatL")
        lf4 = local_d.bitcast(f32r).rearrange("(a x) h -> a (x h)", a=4)
        for q in range(4):
            nc.sync.dma_start(out=flatL[32 * q : 32 * q + 1, :], in_=lf4[q : q + 1])

        # contiguous binary load: [i, (j, c16)], 8KB per partition
        binRaw = persist.tile([128, N * BIN], f32r, tag="binRaw")
        nc.sync.dma_start(out=binRaw, in_=binary_d.bitcast(f32r).rearrange("i j c -> i (j c)"))
        binRaw3 = binRaw.rearrange("p (j c) -> p j c", c=BIN)

        # f32r weights
        wap0 = persist.tile([128, H], f32r, tag="wap0")
        nc.sync.dma_start(out=wap0, in_=wap_d.bitcast(f32r)[0:128])
        wap1 = persist.tile([128, H], f32r, tag="wap1")
        nc.sync.dma_start(out=wap1, in_=wap_d.bitcast(f32r)[128:256])

        biasA = persist.tile([1, H], f32, tag="biasA")
        nc.sync.dma_start(out=biasA, in_=bap_d.unsqueeze(0))
        biasB = persist.tile([1, H], f32, tag="biasB")
        nc.sync.dma_start(out=biasB, in_=bbin_d.unsqueeze(0))
        biasRow = persist.tile([1, H], f32r, tag="biasRow")
        nc.vector.tensor_add(out=biasRow, in0=biasA, in1=biasB)

        wattRow = persist.tile([1, H], f32, tag="wattRow")
        nc.sync.dma_start(out=wattRow, in_=watt_d.rearrange("k o -> o k"))
        battRow = persist.tile([1, 1], f32, tag="battRow")
        nc.sync.dma_start(out=battRow, in_=batt_d.unsqueeze(0))

        # broadcasts: W_att across partitions, b_att column
        wattB = persist.tile([128, H], f32, tag="wattB")
        battCol = persist.tile([128, 1], f32, tag="battCol")
        nc.gpsimd.partition_broadcast(wattB, wattRow)
        nc.gpsimd.partition_broadcast(battCol, battRow)

        # localT = local^T (f32r), then projW = local @ W_apair (f32r)
        localT = persist.tile([128, H], f32r, tag="localT")
        for hb in range(2):
            tp = outpp.tile([128, 2 * H], f32, tag="outp")
            nc.tensor.transpose(tp[:, 0:128], localSb[:, ts(hb, 128)], identity)
            nc.scalar.copy(out=localT[:, ts(hb, 128)], in_=tp[:, 0:128])
        pp = outpp.tile([128, 2 * H], f32, tag="outp")
        nc.tensor.matmul(pp[:, 0:H], lhsT=localT[:, 0:128], rhs=wap0, start=True, stop=False)
        nc.tensor.matmul(pp[:, 0:H], lhsT=localT[:, 128:256], rhs=wap1, start=False, stop=True)
        projW = persist.tile([128, H], f32r, tag="projW")
        nc.scalar.copy(out=projW, in_=pp[:, 0:H])

        # flat proj rows for the attention j-side broadcasts
        projDram = dramp.tile([N, H], f32r, tag="projDram")
        nc.sync.dma_start(out=projDram, in_=projW)
        flatP = persist.tile([97, 32 * H], f32r, tag="flatP")
        pf4 = projDram.rearrange("(a x) h -> a (x h)", a=4)
        for q in range(4):
            nc.sync.dma_start(out=flatP[32 * q : 32 * q + 1, :], in_=pf4[q : q + 1])

        # wx4: W_binary + bias row replicated at partitions {0,32,64,96}
        wx4 = persist.tile([128, H], f32r, tag="wx4")
        for m in range(4):
            nc.sync.dma_start(out=wx4[32 * m : 32 * m + 16, :],
                              in_=wbin_d.bitcast(f32r))
            nc.sync.dma_start(out=wx4[32 * m + 16 : 32 * m + 17, :], in_=biasRow)

        # binp[i, (j, c32)]: c 0..15 = binary[., i?, j, .] (pad-copied),
        # c16 = 1.0 (carries projB row), c17..31 junk (never contracted).
        binp = persist.tile([128, N * CPAD], f32r, tag="binp")
        binp3 = binp.rearrange("p (j c) -> p j c", c=CPAD)
        nc.gpsimd.memset(binp3[:, :, 16:17].bitcast(f32), 1.0)
        for q in range(4):
            nc.gpsimd.tensor_copy(out=binp3[:, ts(q, 32), 0:BIN],
                                  in_=binRaw3[:, ts(q, 32), :])

        binT = {}
        logits = persist.tile([128, N], f32, tag="logits")
        ttrS = persist.tile([128, H], f32, tag="ttrS")
        flatG = persist.tile([97, 32 * H], f32r, tag="flatG")

        # ---------------- helpers ----------------
        def row_rhs(flat, j, width):
            q, r = divmod(j, 32)
            return flat[32 * q : 32 * q + 1, r * H : r * H + width]

        def row_lhsT(j):
            q = j // 32
            return onesT[32 * q : 32 * q + 1, :]

        def row_tp(j):
            return (32 * (j // 32), 0)

        def bc_pair(j, xSb, flat, stage):
            # stage[:, j%JB .. +2 tiles] = x[i,:] + x[j..j+1,:]
            dst = stage[:, (j % JB) * H : (j % JB) * H + 2 * H]
            po = bcp.tile([128, 2 * H], f32, tag="bc")
            nc.tensor.matmul(po, lhsT=row_lhsT(j), rhs=row_rhs(flat, j, 2 * H),
                             start=True, stop=True, tile_position=row_tp(j))
            nc.vector.tensor_add(
                out=dst, in0=xSb.unsqueeze(1).broadcast_to([128, 2, H]), in1=po)

        def transpose_block(k):
            # binp free cols [128k, 128k+128) = j 4k..4k+3 x c32
            tp = outpp.tile([128, 2 * H], f32, tag="outp")
            nc.tensor.transpose(tp[:, 0:128].bitcast(f32r), binp[:, ts(k, 128)], identR)
            bt = persist.tile([128, 128], f32r, tag=f"binT{k}")
            nc.scalar.copy(out=bt, in_=tp[:, 0:128])
            binT[k] = bt

        def attn_pair(j):
            k = j // 4
            if j % 4 == 0:
                transpose_block(k)
            if ATTN_LEVEL < 2:
                return
            pre = prep.tile([128, 2 * H], f32, tag="pre")
            nc.tensor.matmul(pre, lhsT=row_lhsT(j), rhs=row_rhs(flatP, j, 2 * H),
                             start=True, stop=False, tile_position=row_tp(j))
            for m in range(2):
                jl = (j + m) % 4
                nc.tensor.matmul(pre[:, ts(m, H)], lhsT=identR, rhs=projW,
                                 start=False, stop=False)
                nc.tensor.matmul(
                    pre[:, ts(m, H)],
                    lhsT=binT[k][32 * jl : 32 * jl + 17, :],
                    rhs=wx4[32 * jl : 32 * jl + 17, :],
                    start=False, stop=(m == 1), tile_position=(32 * jl, 0),
                )
            a2 = att2p.tile([128, 2 * H], f32, tag="att2")
            nc.scalar.activation(out=a2, in_=pre, func=Relu)
            for m in range(2):
                nc.vector.affine_mul_reduce(
                    out=ttrS, accum_out=logits[:, j + m : j + m + 1],
                    in0=a2[:, ts(m, H)], in1=wattB, scale=1.0, bias=0.0,
                )

        def flush(j0, stage, dram_out):
            nc.sync.dma_start(
                out=dram_out[:, j0 : j0 + JB, :],
                in_=stage.rearrange("p (j h) -> p j h", h=H),
            )

        # ---------------- phase 1: local_pair + attention ----------------
        stage = None
        for j in range(0, N, 2):
            if j % JB == 0:
                stage = stagep.tile([128, JB * H], f32, tag="stage")
            bc_pair(j, localSb, flatL, stage)
            if not SKIP_ATTN:
                attn_pair(j)
            if (j + 2) % JB == 0:
                flush(j + 2 - JB, stage, lp_d)

        # ---------------- scores -> glob ----------------
        globSb = persist.tile([128, H], f32, tag="globSb")
        globR = persist.tile([128, H], f32r, tag="globR")
        if SKIP_ATTN or ATTN_LEVEL < 3:
            nc.vector.tensor_copy(out=globSb, in_=localSb)
            nc.vector.tensor_copy(out=globR, in_=localSb)
        else:
            score = persist.tile([128, N], f32, tag="score")
            nc.scalar.activation(out=score, in_=logits, func=Sigmoid, bias=battCol)
            tp = outpp.tile([128, 2 * H], f32, tag="outp")
            nc.tensor.transpose(tp[:, 0:128], score, identity)
            scoreTS = persist.tile([128, N], f32, tag="scoreTS")
            nc.vector.tensor_copy(out=scoreTS, in_=tp[:, 0:128])
            pg = outpp.tile([128, 2 * H], f32, tag="outp")
            nc.tensor.matmul(pg[:, 0:H], lhsT=scoreTS, rhs=localSb, start=True, stop=True)
            nc.vector.tensor_copy(out=globSb, in_=pg[:, 0:H])
            nc.scalar.copy(out=globR, in_=pg[:, 0:H])
        globDramR = dramp.tile([N, H], f32r, tag="globDramR")
        nc.sync.dma_start(out=globDramR, in_=globR)
        gf4 = globDramR.rearrange("(a x) h -> a (x h)", a=4)
        for q in range(4):
            nc.sync.dma_start(out=flatG[32 * q : 32 * q + 1, :], in_=gf4[q : q + 1])

        # ---------------- phase 2: global_pair ----------------
        stage = None
        for j in range(0, N, 2):
            if j % JB == 0:
                stage = stagep.tile([128, JB * H], f32, tag="stage")
            bc_pair(j, globSb, flatG, stage)
            if (j + 2) % JB == 0:
                flush(j + 2 - JB, stage, gp_d)


def _build(reps=1):
    import concourse.bass as bass  # noqa: F401
    from concourse import bacc
    import concourse.mybir as mybir
    import concourse.tile as tile

    f32 = mybir.dt.float32
    nc = bacc.Bacc(
        "TRN2",
        target_bir_lowering=False,
        debug=False,
        enable_asserts=False,
        num_devices=NCORES,
    )
    io = (
        nc.dram_tensor("local", [N, H], f32, kind="ExternalInput").ap(),
        nc.dram_tensor("binary", [N, N, BIN], f32, kind="ExternalInput").ap(),
        nc.dram_tensor("w_apair", [H, H], f32, kind="ExternalInput").ap(),
        nc.dram_tensor("b_apair", [H], f32, kind="ExternalInput").ap(),
        nc.dram_tensor("w_binary", [BIN, H], f32, kind="ExternalInput").ap(),
        nc.dram_tensor("b_binary", [H], f32, kind="ExternalInput").ap(),
        nc.dram_tensor("w_att", [H, 1], f32, kind="ExternalInput").ap(),
        nc.dram_tensor("b_att", [1], f32, kind="ExternalInput").ap(),
        nc.dram_tensor("out_lp", [N, N, H], f32, kind="ExternalOutput").ap(),
        nc.dram_tensor("out_gp", [N, N, H], f32, kind="ExternalOutput").ap(),
    )
    with tile.TileContext(nc) as tc:
        _body(tc, io, reps=reps)
    nc.compile()
    return nc


def _get_nc():
    if "nc" not in _cache:
        _cache["nc"] = _build()
    return _cache["nc"]


def _run(inputs, trace=False):
    from concourse.bass_utils import run_bass_kernel_spmd

    nc = _get_nc()
    f = lambda x: np.ascontiguousarray(np.asarray(x), dtype=np.float32)
    shared = {
        "w_apair": f(inputs["W_apair"]),
        "b_apair": f(inputs["b_apair"]),
        "w_binary": f(inputs["W_binary"]),
        "b_binary": f(inputs["b_binary"]),
        "w_att": f(inputs["W_att"]),
        "b_att": f(inputs["b_att"]),
    }
    local = f(inputs["local_feats"])
    binary = f(inputs["binary_feats"])
    in_maps = [
        {"local": local[c], "binary": binary[c], **shared} for c in range(NCORES)
    ]
    res = run_bass_kernel_spmd(
        nc, in_maps, core_ids=list(range(NCORES)), trace=trace
    )
    lp = np.stack([r["out_lp"] for r in res.results])
    gp = np.stack([r["out_gp"] for r in res.results])
    return (lp, gp), res


def kernel(**inputs):
    out, _ = _run(inputs, trace=False)
    return out
